# revision 27
# baseline (speedup 1.0000x reference)
"""MMoE layer kernel for 8 Trainium2 NeuronCores.

Reference math (B=4096, D=1024, H1=2048, H2=1024, E=7 experts, NS=7 scenes):
  h        = relu(einsum('bd,edh', x, W1) + b1)           # [B,E,H1]
  eo       = relu(einsum('beh,eho', h, W2) + b2)          # [B,E,H2]
  xc       = concat(x, scene_emb[scene])                  # [B, D+16]
  G        = softmax over s of einsum('bd,sde', xc, S)    # [B,E,NS] (after transpose)
  q        = mean_s log(G*7)                              # [B,E]
  score1   = logG[b, e, scene_b]
  select   = drop expert e iff e == argmin_e score1 == argmin_e q
  gate     = softmax_e(G[b,e,scene_b]) * select
  out      = einsum('be,beo', gate, eo); output = stack([out, out])

Sharding: data-parallel over batch (512 rows/core), weights replicated.
Expert MLP matmuls run in bf16 (fp32 accumulation in PSUM); all routing
math stays fp32 so the argmin/select decisions are bit-stable. (fp8
DoubleRow was evaluated and rejected: e4m3 expert MLPs measure
2.7e-2..3.8e-2 rel error vs the 2e-2 gate, and every residual-correction
scheme costs as many extra matmuls as DoubleRow saves.)

Schedule (from perfetto/NTFF analysis; the kernel is PE-bound at ~96%
occupancy, 1792 N=512 bf16 matmuls x 215.8ns warm):
  - DMA queue order = critical path: xtb[kt0-3], w1(0)[m0,kt0-3] (the
    0.625MB gating L1(0)'s first matmul group), rest of xtb/w1(0) in
    per-m-tile chunks, routing inputs, then w2/w1 per expert.
  - w1 rides in a host-permuted chunk-major layout so every chunk is a
    contiguous per-partition DMA (strided chunks cost 1.8-2.8us of
    sync-engine issue time each, measured).
  - 36 cold-clock warm-up matmuls bridge the ~6.5us first-MM->DMA-ready
    window; undershooting lets the HAM clock gate re-throttle L1(0) to
    1.2GHz (measured 4.6us loss).
  - The fp32 routing matmuls (N=49, ~45% PE duty) interleave into
    L1(0)'s tail one b-tile per two m-groups — a contiguous block of
    them drops PE activity enough that HAM re-throttles (measured 10us
    of half-clock L2(0)).
  - Tail: per-half-row output DMAs; the last row's evacuation is
    pipelined in 256-col halves (act/add/DMA overlap).

Device decomposition of the routing (no cross-partition broadcasts):
  Gpre[b, e*7+s] = x[b] @ Sflat + SE_table[scene_b]   (SE_table = scene_emb @ S[:,D:,:])
  Z = sum_s exp(Gpre); logZ = ln Z; SG = sum_s Gpre
  q      = SG/7 - logZ            (+const, argmin only)
  score1 = sum_s Gpre*onehot_s(scene) - logZ
  gate0  = softmax_e(exp(score1)) (logits in (0,1): no max-subtract needed)
  sel    = 1 - ismin(score1)*ismin(q)
  gate   = gate0 * sel
"""

import sys

if "/opt/trn_rl_repo" not in sys.path:
    sys.path.insert(0, "/opt/trn_rl_repo")

from contextlib import ExitStack

import ml_dtypes
import numpy as np

import concourse.bass as bass
import concourse.tile as tile
from concourse import bacc, mybir
from concourse.bass_utils import run_bass_kernel_spmd

F32 = mybir.dt.float32
BF16 = mybir.dt.bfloat16
AF = mybir.ActivationFunctionType
ALU = mybir.AluOpType
AX = mybir.AxisListType

N_CORES = 8
B, D, H1, H2, E, NS, T = 4096, 1024, 2048, 1024, 7, 7, 2
BL = B // N_CORES          # 512 rows per core
NB = BL // 128             # 4 batch tiles
KT1 = D // 128             # 8  k-tiles, layer 1
MT1 = H1 // 128            # 16 m-tiles, layer 1
KT2 = H1 // 128            # 16 k-tiles, layer 2
NO = H2 // 512             # 2  512-wide out column blocks
EN = E * NS                # 49
NP_BF16 = np.dtype(ml_dtypes.bfloat16)


def _emit_kernel(tc, aps, has_b1, has_b2):
    nc = tc.nc
    ctx = ExitStack()
    with ctx:
        # Pool stack order matters: the expert-weight pools are allocated
        # BEFORE the routing pool so they never reuse the routing pool's
        # released SBUF addresses — otherwise Tile serializes the first
        # weight DMAs behind every routing matmul (measured 13µs PE stall).
        consts = ctx.enter_context(tc.tile_pool(name="consts", bufs=1))
        w1pool = ctx.enter_context(tc.tile_pool(name="w1", bufs=2))
        w2pool = ctx.enter_context(tc.tile_pool(name="w2", bufs=1))
        htpool = ctx.enter_context(tc.tile_pool(name="ht", bufs=1))
        tmppool = ctx.enter_context(tc.tile_pool(name="tmp", bufs=3))
        l1ps = ctx.enter_context(tc.tile_pool(name="l1ps", bufs=4, space="PSUM"))
        l2ps = ctx.enter_context(tc.tile_pool(name="l2ps", bufs=4, space="PSUM"))
        rpool = tc.alloc_tile_pool(name="routing", bufs=1)

        # ---- PE warm-up: dummy matmuls from memset tiles (no input deps)
        # fill the DMA boot window (~5µs boot + ~4µs until w1(0)'s first
        # chunk lands) and flip the HAM clock gate to 8/8 before layer 1 of
        # expert 0 starts. 20 × N=256 ≈ 4.3µs at the cold clock. -----------
        warm_sb = rpool.tile([128, 256], BF16)
        nc.vector.memset(warm_sb[:, :], 0.0)
        # 36 × N=256 ≈ 6.8µs at the cold clock — matched to the measured
        # first-MM→DMA-ready latency (~6.5µs, boot-shift-invariant). Under-
        # shooting idles the PE and the HAM gate re-throttles L1 to half
        # clock (measured 4.6µs loss); overshooting costs only ~0.1µs/MM.
        warm_ps = l1ps.tile([128, 256], F32, tag="ps1", name="warm_ps")
        for _ in range(36):
            nc.tensor.matmul(
                warm_ps[:, :], lhsT=warm_sb[:, 0:128], rhs=warm_sb[:, :],
                start=True, stop=True,
            )

        # ---- critical-path DMAs lead the sync queue: xTb then w1(0) in
        # chunk-major layout (host pre-permutes W1 so each chunk is a
        # contiguous 4KB-per-partition transfer — strided m-column chunks
        # cost 1.8-2.8µs of sync-engine issue time EACH, measured).
        # L1(0)'s first matmuls are then gated on ~1.5MB of traffic
        # (~13.5µs incl. the 7µs queue boot) instead of the full routing
        # stream. Routing inputs follow — the routing matmuls now run
        # interleaved into L1(0)'s tail, by which point xT has landed. ----
        # w1 DRAM layout: [E, 128, m, kt, j] with one chunk per m-tile,
        # lhsT(kt, m) = w1_sb[:, m, kt, :]. The first L1 matmul group's
        # k-half needs only xtb[kt0-3] + w1[m0, kt0-3] = 0.625MB, so those
        # two transfers lead; everything else streams behind while the PE
        # works. 16 warm-up matmuls (~3.4µs cold = one HAM window) bridge
        # the remaining gap regardless of how late the DMA ring boots.
        xtb_sb = consts.tile([128, KT1, BL], BF16)
        xtb_src = aps["xTb"].rearrange("(t p) b -> p t b", p=128)
        w1_e0 = w1pool.tile([128, MT1, KT1, 128], BF16, tag="w1")
        w1_e0_src = aps["w1"][0].rearrange("p (c k j) -> p c k j", c=MT1, k=KT1)
        nc.sync.dma_start(xtb_sb[:, 0 : KT1 // 2, :], xtb_src[:, 0 : KT1 // 2, :])
        nc.sync.dma_start(w1_e0[:, 0, 0:4, :], w1_e0_src[:, 0, 0:4, :])
        nc.sync.dma_start(xtb_sb[:, KT1 // 2 :, :], xtb_src[:, KT1 // 2 :, :])
        nc.sync.dma_start(w1_e0[:, 0, 4:8, :], w1_e0_src[:, 0, 4:8, :])
        for a, b in ((1, 2), (2, 4), (4, 8), (8, 12), (12, 16)):
            nc.sync.dma_start(w1_e0[:, a:b, :, :], w1_e0_src[:, a:b, :, :])

        sflat_sb = rpool.tile([128, KT1, EN], F32)
        nc.sync.dma_start(sflat_sb[:, :, :], aps["sflat"].rearrange("(t p) j -> p t j", p=128))
        xt_sb = rpool.tile([128, KT1, BL], F32)
        xt_src = aps["xT"].rearrange("(t p) b -> p t b", p=128)
        for qq in range(2):
            nc.sync.dma_start(
                xt_sb[:, 4 * qq : 4 * qq + 4, :], xt_src[:, 4 * qq : 4 * qq + 4, :]
            )
        sett_sb = rpool.tile([10, EN], F32)
        nc.sync.dma_start(sett_sb[:, :], aps["sett"][:, :])
        scolr_sb = rpool.tile([128, NB * EN], F32)
        nc.sync.dma_start(scolr_sb[:, :], aps["scol_rep"][:, :])
        srow10_sb = rpool.tile([10, BL], F32)
        nc.sync.dma_start(srow10_sb[:, :], aps["srow"].to_broadcast((10, BL)))
        io7_sb = rpool.tile([128, NB * EN], F32)
        nc.sync.dma_start(io7_sb[:, :], aps["iota7"].to_broadcast((128, NB * EN)))
        io10_sb = rpool.tile([10, 1], F32)
        nc.sync.dma_start(io10_sb[:, :], aps["iota10"][:, :])

        gate_sb = consts.tile([128, NB, E], F32)
        acc_sb = consts.tile([128, NB, H2], F32)
        if has_b1:
            b1_sb = consts.tile([128, E * MT1], F32)
            nc.sync.dma_start(b1_sb[:, :], aps["b1t"][:, :])
        if has_b2:
            b2_sb = consts.tile([1, E * H2], BF16)
            nc.sync.dma_start(b2_sb[:, :], aps["b2f"][:, :])
            ones_sb = consts.tile([1, 128], BF16)
            nc.vector.memset(ones_sb[:, :], 1.0)

        # onehot over embedding rows, [10, BL]: onehot[r, b] = (scene[b] == r)
        onehot_sb = rpool.tile([10, BL], F32)
        nc.vector.tensor_scalar(
            out=onehot_sb[:, :], in0=srow10_sb[:, :],
            scalar1=io10_sb[:, 0:1], scalar2=None, op0=ALU.is_equal,
        )

        gp = rpool.tile([128, NB * EN], F32)  # all 4 b-tiles side by side

        def routing_matmuls(t):
            """Gpre matmuls for b-tile t. Interleaved into L1(0)'s m-loop
            tail (one t-group per two m-groups) so the PE's activity stays
            dense — a contiguous block of these short N=49 matmuls is
            ~43% PE-duty and makes the HAM clock-gate re-throttle to 4/8
            (measured: 10µs of half-clock L2(0)). psr tiles live in the
            (still unused) L2 PSUM slots and are copied out before L2(0)'s
            rotation reaches them."""
            psr_t = l2ps.tile([128, EN], F32, tag="ps2", name=f"psr{t}")
            for kt in range(KT1):
                nc.tensor.matmul(
                    psr_t[:, :],
                    lhsT=xt_sb[:, kt, bass.ts(t, 128)],
                    rhs=sflat_sb[:, kt, :],
                    start=(kt == 0), stop=False,
                )
            nc.tensor.matmul(
                psr_t[:, :],
                lhsT=onehot_sb[:, bass.ts(t, 128)],
                rhs=sett_sb[:, :],
                start=False, stop=True,
            )
            nc.scalar.copy(gp[:, bass.ts(t, EN)], psr_t[:, :])

        def routing_chain():
            """Gate computation, fused over all 4 b-tiles ([128, 4*49]).

            Emitted AFTER layer 1 of expert 0: the scalar engine's queue is
            strict FIFO, so emitting this serial chain before the L1 PSUM
            evacuations would block them (and stall the PE on PSUM slots).
            The gate is only consumed by expert 0's layer-2 evacuation.
            """
            NE = NB * E  # 28
            gp4 = gp.rearrange("p (t e s) -> p (t e) s", s=NS, e=E)
            eex = rpool.tile([128, NB * EN], F32)
            nc.scalar.activation(eex[:, :], gp[:, :], AF.Exp)
            z = rpool.tile([128, NE], F32)
            nc.vector.tensor_reduce(out=z[:, :], in_=eex.rearrange("p (t e s) -> p (t e) s", s=NS, e=E), axis=AX.X, op=ALU.add)
            logz = rpool.tile([128, NE], F32)
            nc.scalar.activation(logz[:, :], z[:, :], AF.Ln)
            sg = rpool.tile([128, NE], F32)
            nc.vector.tensor_reduce(out=sg[:, :], in_=gp4, axis=AX.X, op=ALU.add)
            q = rpool.tile([128, NE], F32)
            nc.vector.scalar_tensor_tensor(
                out=q[:, :], in0=sg[:, :], scalar=1.0 / NS, in1=logz[:, :],
                op0=ALU.mult, op1=ALU.subtract,
            )
            oh = rpool.tile([128, NB * EN], F32)
            nc.vector.tensor_tensor(out=oh[:, :], in0=io7_sb[:, :], in1=scolr_sb[:, :], op=ALU.is_equal)
            gsel = rpool.tile([128, NB * EN], F32)
            nc.vector.tensor_tensor(out=gsel[:, :], in0=gp[:, :], in1=oh[:, :], op=ALU.mult)
            s1s = rpool.tile([128, NE], F32)
            nc.vector.tensor_reduce(out=s1s[:, :], in_=gsel.rearrange("p (t e s) -> p (t e) s", s=NS, e=E), axis=AX.X, op=ALU.add)
            score1 = rpool.tile([128, NE], F32)
            nc.vector.tensor_tensor(out=score1[:, :], in0=s1s[:, :], in1=logz[:, :], op=ALU.subtract)

            lg = rpool.tile([128, NE], F32)
            nc.scalar.activation(lg[:, :], score1[:, :], AF.Exp)     # G at scene, in (0,1)
            el = rpool.tile([128, NE], F32)
            nc.scalar.activation(el[:, :], lg[:, :], AF.Exp)         # softmax numerator
            # per-b-tile scalars ([128,1]) for the reductions' broadcasts
            ssum = rpool.tile([128, NB], F32)
            rs = rpool.tile([128, NB], F32)
            m1 = rpool.tile([128, NB], F32)
            m2 = rpool.tile([128, NB], F32)
            k1 = rpool.tile([128, NE], F32)
            k2 = rpool.tile([128, NE], F32)
            g0 = rpool.tile([128, NE], F32)
            el3 = el.rearrange("p (t e) -> p t e", e=E)
            sc3 = score1.rearrange("p (t e) -> p t e", e=E)
            q3 = q.rearrange("p (t e) -> p t e", e=E)
            nc.vector.tensor_reduce(out=ssum[:, :], in_=el3, axis=AX.X, op=ALU.add)
            nc.vector.reciprocal(rs[:, :], ssum[:, :])
            nc.vector.tensor_reduce(out=m1[:, :], in_=sc3, axis=AX.X, op=ALU.min)
            nc.vector.tensor_reduce(out=m2[:, :], in_=q3, axis=AX.X, op=ALU.min)
            for t in range(NB):
                nc.vector.tensor_scalar(
                    out=k1[:, bass.ts(t, E)], in0=score1[:, bass.ts(t, E)],
                    scalar1=m1[:, t : t + 1], scalar2=None, op0=ALU.is_equal,
                )
                nc.vector.tensor_scalar(
                    out=k2[:, bass.ts(t, E)], in0=q[:, bass.ts(t, E)],
                    scalar1=m2[:, t : t + 1], scalar2=None, op0=ALU.is_equal,
                )
                nc.vector.tensor_scalar(
                    out=g0[:, bass.ts(t, E)], in0=el[:, bass.ts(t, E)],
                    scalar1=rs[:, t : t + 1], scalar2=None, op0=ALU.mult,
                )
            kill = rpool.tile([128, NE], F32)
            nc.vector.tensor_tensor(out=kill[:, :], in0=k1[:, :], in1=k2[:, :], op=ALU.mult)
            sel = rpool.tile([128, NE], F32)
            nc.vector.tensor_scalar(
                out=sel[:, :], in0=kill[:, :], scalar1=-1.0, scalar2=1.0,
                op0=ALU.mult, op1=ALU.add,
            )
            gate_flat = gate_sb.rearrange("p t e -> p (t e)")
            nc.vector.tensor_tensor(out=gate_flat[:, :], in0=g0[:, :], in1=sel[:, :], op=ALU.mult)

        # ---- expert MLPs (bf16 matmuls, fp32 accumulation) -------------
        for e in range(E):
            # All bulk traffic rides the sync HWDGE queue in program order —
            # the per-core DMA fabric saturates at ~350GB/s regardless of
            # queue count, so ordering (not parallel queues) is what matters.
            # Two half-DMAs per weight: one trigger splits across all 16 SDMA
            # engines, and halves complete earlier than one monolithic sem.
            if e == 0:
                w1_sb = w1_e0  # DMA'd at the head of the queue, in m-chunks
            else:
                w1_sb = w1pool.tile([128, MT1, KT1, 128], BF16, tag="w1")
                w1_src = aps["w1"][e].rearrange("p (c k j) -> p c k j", c=MT1, k=KT1)
                nc.sync.dma_start(w1_sb[:, 0:8, :, :], w1_src[:, 0:8, :, :])
                nc.sync.dma_start(w1_sb[:, 8:16, :, :], w1_src[:, 8:16, :, :])
            w2_sb = w2pool.tile([128, KT2, H2], BF16, tag="w2")
            w2_src = aps["w2"][e].rearrange("(t p) o -> p t o", p=128)
            nc.sync.dma_start(w2_sb[:, 0 : KT2 // 2, :], w2_src[:, 0 : KT2 // 2, :])
            nc.sync.dma_start(w2_sb[:, KT2 // 2 :, :], w2_src[:, KT2 // 2 :, :])

            # layer 1: hT[f, b] = relu(sum_d W1[d, f] * xT[d, b] + b1[f])
            ht_sb = htpool.tile([128, KT2, BL], BF16, tag="ht")
            for m in range(MT1):
                ps = l1ps.tile([128, BL], F32, tag="ps1")
                for kt in range(KT1):
                    nc.tensor.matmul(
                        ps[:, :],
                        lhsT=w1_sb[:, m, kt, :],
                        rhs=xtb_sb[:, kt, :],
                        start=(kt == 0), stop=(kt == KT1 - 1),
                    )
                bias1 = b1_sb[:, e * MT1 + m : e * MT1 + m + 1] if has_b1 else 0.0
                nc.scalar.activation(ht_sb[:, m, :], ps[:, :], AF.Relu, bias=bias1)
                if e == 0 and m >= 9 and m % 2 == 1:
                    # xT has landed by m=9 (~30µs); one t-group per two
                    # m-groups keeps PE duty ~85% through the tail.
                    routing_matmuls((m - 9) // 2)

            if e == 0:
                # The gate math slots in after L1(0)'s evacuations on the
                # scalar queue and completes well before L2(0)'s first
                # evacuation needs it.
                routing_chain()
                rpool.release()

            # layer 2: out[b, o] = relu(sum_h hT[h, b] * W2[h, o] + b2[o])
            for mb in range(NB):
                for no in range(NO):
                    ps2 = l2ps.tile([128, 512], F32, tag="ps2")
                    for kt in range(KT2):
                        nc.tensor.matmul(
                            ps2[:, :],
                            lhsT=ht_sb[:, kt, bass.ts(mb, 128)],
                            rhs=w2_sb[:, kt, bass.ts(no, 512)],
                            start=(kt == 0),
                            stop=(kt == KT2 - 1 and not has_b2),
                        )
                    if has_b2:
                        nc.tensor.matmul(
                            ps2[:, :],
                            lhsT=ones_sb[:, :],
                            rhs=b2_sb[:, e * H2 + no * 512 : e * H2 + (no + 1) * 512],
                            start=False, stop=True,
                        )
                    gcol = gate_sb[:, mb, e : e + 1]
                    last_unit = e == E - 1 and mb == NB - 1 and no == NO - 1
                    if e == 0:
                        nc.scalar.activation(
                            acc_sb[:, mb, bass.ts(no, 512)], ps2[:, :], AF.Relu, scale=gcol
                        )
                    elif last_unit:
                        # The very last evacuation is on the kernel's tail:
                        # pipeline it in 256-col halves (act/add/DMA of half
                        # 0 overlap act/add of half 1).
                        for hh in range(2):
                            sl = slice(no * 512 + hh * 256, no * 512 + (hh + 1) * 256)
                            tmp = tmppool.tile([128, 256], F32, tag="tmp")
                            nc.scalar.activation(
                                tmp[:, :], ps2[:, hh * 256 : (hh + 1) * 256],
                                AF.Relu, scale=gcol,
                            )
                            nc.vector.tensor_tensor(
                                out=acc_sb[:, mb, sl], in0=acc_sb[:, mb, sl],
                                in1=tmp[:, :], op=ALU.add,
                            )
                            nc.sync.dma_start(
                                aps["out"].rearrange("(t p) o -> p t o", p=128)[:, mb, sl],
                                acc_sb[:, mb, sl],
                            )
                    else:
                        tmp = tmppool.tile([128, 512], F32, tag="tmp")
                        nc.scalar.activation(tmp[:, :], ps2[:, :], AF.Relu, scale=gcol)
                        nc.vector.tensor_tensor(
                            out=acc_sb[:, mb, bass.ts(no, 512)],
                            in0=acc_sb[:, mb, bass.ts(no, 512)],
                            in1=tmp[:, :], op=ALU.add,
                        )
                    # Per-half-tile output DMA so each 256KB store starts as
                    # soon as its accumulator half is final — the kernel's
                    # tail only pays for the last quarter-row.
                    if e == E - 1 and not last_unit:
                        nc.sync.dma_start(
                            aps["out"].rearrange("(t p) o -> p t o", p=128)[
                                :, mb, no * 512 : (no + 1) * 512
                            ],
                            acc_sb[:, mb, bass.ts(no, 512)],
                        )


def build(has_b1, has_b2):
    """Build + schedule + compile the Bass program. Returns nc."""
    nc = bacc.Bacc("TRN2", target_bir_lowering=False, debug=False)
    aps = {}
    aps["xT"] = nc.dram_tensor("xT", [D, BL], F32, kind="ExternalInput").ap()
    aps["xTb"] = nc.dram_tensor("xTb", [D, BL], BF16, kind="ExternalInput").ap()
    aps["w1"] = nc.dram_tensor("w1", [E, 128, D * H1 // 128], BF16, kind="ExternalInput").ap()
    aps["w2"] = nc.dram_tensor("w2", [E, H1, H2], BF16, kind="ExternalInput").ap()
    if has_b1:
        aps["b1t"] = nc.dram_tensor("b1t", [128, E * MT1], F32, kind="ExternalInput").ap()
    if has_b2:
        aps["b2f"] = nc.dram_tensor("b2f", [1, E * H2], BF16, kind="ExternalInput").ap()
    aps["sflat"] = nc.dram_tensor("sflat", [D, EN], F32, kind="ExternalInput").ap()
    aps["sett"] = nc.dram_tensor("sett", [10, EN], F32, kind="ExternalInput").ap()
    aps["scol_rep"] = nc.dram_tensor("scol_rep", [128, NB * EN], F32, kind="ExternalInput").ap()
    aps["srow"] = nc.dram_tensor("srow", [1, BL], F32, kind="ExternalInput").ap()
    aps["iota7"] = nc.dram_tensor("iota7", [1, NB * EN], F32, kind="ExternalInput").ap()
    aps["iota10"] = nc.dram_tensor("iota10", [10, 1], F32, kind="ExternalInput").ap()
    aps["out"] = nc.dram_tensor("out", [BL, H2], F32, kind="ExternalOutput").ap()

    with tile.TileContext(nc) as tc:
        _emit_kernel(tc, aps, has_b1, has_b2)
    nc.compile()
    return nc


def make_in_maps(inputs):
    """Host-side layout prep + batch sharding. Returns (in_maps, has_b1, has_b2)."""
    x = np.ascontiguousarray(np.asarray(inputs["x"], dtype=np.float32))
    scene = np.asarray(inputs["scene"]).astype(np.int64)
    W1 = np.asarray(inputs["W1"], dtype=np.float32)
    b1 = np.asarray(inputs["b1"], dtype=np.float32)
    W2 = np.asarray(inputs["W2"], dtype=np.float32)
    b2 = np.asarray(inputs["b2"], dtype=np.float32)
    S = np.asarray(inputs["S"], dtype=np.float32)
    scene_emb = np.asarray(inputs["scene_emb"], dtype=np.float32)

    has_b1 = bool(np.any(b1))
    has_b2 = bool(np.any(b2))

    # chunk-major layout: w1b[e, p, m, kt, j] = W1[e, kt*128+p, m*128+j] so
    # each m-tile chunk is one contiguous 2KB-per-partition DMA.
    w1b = np.ascontiguousarray(
        W1.astype(NP_BF16)
        .reshape(E, KT1, 128, MT1, 128)
        .transpose(0, 2, 3, 1, 4)
        .reshape(E, 128, D * H1 // 128)
    )
    w2b = np.ascontiguousarray(W2.astype(NP_BF16))
    sflat = np.ascontiguousarray(S[:, :D, :].transpose(1, 2, 0).reshape(D, EN))
    sett = np.ascontiguousarray(
        np.einsum("rm,sme->res", scene_emb, S[:, D:, :]).reshape(scene_emb.shape[0], EN)
    )
    iota7 = np.tile(np.arange(EN, dtype=np.float32) % NS, NB).reshape(1, NB * EN)
    iota10 = np.arange(10, dtype=np.float32).reshape(10, 1)
    shared = {
        "w1": w1b, "w2": w2b, "sflat": sflat, "sett": sett,
        "iota7": iota7, "iota10": iota10,
    }
    if has_b1:
        shared["b1t"] = np.ascontiguousarray(
            b1.reshape(E, MT1, 128).transpose(2, 0, 1).reshape(128, E * MT1)
        )
    if has_b2:
        shared["b2f"] = np.ascontiguousarray(b2.astype(NP_BF16).reshape(1, E * H2))

    in_maps = []
    for c in range(N_CORES):
        xs = x[c * BL : (c + 1) * BL]
        sc = scene[c * BL : (c + 1) * BL]
        xT = np.ascontiguousarray(xs.T)
        m = dict(shared)
        m["xT"] = xT
        m["xTb"] = np.ascontiguousarray(xT.astype(NP_BF16))
        scol = sc.reshape(NB, 128).T.astype(np.float32)          # [128, NB]
        m["scol_rep"] = np.ascontiguousarray(
            np.repeat(scol[:, :, None], EN, axis=2).reshape(128, NB * EN)
        )
        m["srow"] = np.ascontiguousarray(sc.astype(np.float32).reshape(1, BL))
        in_maps.append(m)
    return in_maps, has_b1, has_b2


_NC_CACHE = {}


def get_compiled(has_b1, has_b2):
    key = (has_b1, has_b2)
    if key not in _NC_CACHE:
        _NC_CACHE[key] = build(has_b1, has_b2)
    return _NC_CACHE[key]


def run(inputs, trace=False, **kwargs):
    """Run on hardware; returns (full_output, BassKernelResults)."""
    in_maps, has_b1, has_b2 = make_in_maps(inputs)
    nc = get_compiled(has_b1, has_b2)
    res = run_bass_kernel_spmd(nc, in_maps, core_ids=list(range(N_CORES)), trace=trace, **kwargs)
    parts = [res.results[c]["out"] for c in range(N_CORES)]
    out = np.concatenate(parts, axis=0).astype(np.float32)
    full = np.ascontiguousarray(np.broadcast_to(out[None], (T, B, H2)))
    return full, res


def kernel(**inputs):
    full, _ = run(inputs, trace=False)
    return full

